# revision 3
# baseline (speedup 1.0000x reference)
"""Trainium2 Bass kernel for AttentionSocialPooling.

Data-parallel over batch B=8 (one NeuronCore per batch element); each
core runs T=64 independent NxN social-pooling steps.

Structure (vs baseline): per t the H matmul emits 16 channels y_k =
e_k*(u_k[i]+v_k[j]) channel-major.  ACT evacs slots 0:8 in ONE Relu
(stored |contrib|, sign sigma tracked host-side); DVE evacs slots 8:16
with scalar_tensor_tensor ops that FUSE the pair-reduction L1:
Q_k = (y_dve op0 0) op1 R_act[k]  (op0 by own sign, op1 by partner
sigma).  The 8 signed partials Q are then summed ON THE PE via 8
copy-accumulate matmuls (identity stationary) into PSUM S_pe; the att
sigmoid reads S_pe directly from PSUM.  No DVE tree at all.

The distance mask is a steep sigmoid on ACT (m = sigmoid(4*z),
z = 64*(R^2-d^2) from the hi/lo fp16 dist matmul) instead of a DVE
clip; w = att*m on GPSIMD (otherwise idle).  Finals + tail as baseline:
deferred w/m row-sum matmuls into pf, per-8t tail divides by count-1.
"""

import numpy as np
import ml_dtypes

B, T, N, C, A = 8, 64, 128, 2, 16
R2 = 2500.0
KAPPA = 16.0
NA = N * A             # 2048 columns per t
NAZ = NA + N           # + 128 z columns (dist folded into the H matmul)
HALF = NA // 2
TG = 8                 # t-group size for tail batching
NG = T // TG

bf16 = ml_dtypes.bfloat16
f16 = np.float16

_CACHE = {}


def _plan_channels(w2):
    """Slots 0:8 = ACT channels (stored |contrib|, e=|w2|), slots 8:16 =
    DVE channels (stored signed contrib, e=w2).  DVE slot k pairs with
    ACT slot k.  Returns (order, emis, groups) where groups is a list of
    (start, end, op0, op1) for the DVE STT ops over slot range [s,e).
    """
    pos = [int(i) for i in np.where(w2 >= 0)[0]]
    neg = [int(i) for i in np.where(w2 < 0)[0]]
    P, Nn = len(pos), len(neg)
    assert P + Nn == A

    if P >= 8:
        act = pos[:8]                       # all sigma=+1 -> op1=add
        dve = pos[8:] + neg                 # [max-group | min-group]
        groups = []
        if P - 8 > 0:
            groups.append((0, P - 8, "max", "add"))
        if Nn > 0:
            groups.append((P - 8, 8, "min", "add"))
    else:
        act = pos + neg[8 - P:]             # [sigma+ | sigma-]
        dve = neg[:8]                       # all min
        groups = []
        if P > 0:
            groups.append((0, P, "min", "add"))
        groups.append((P, 8, "min", "subtract"))

    order = act + dve
    assert len(order) == A and sorted(order) == list(range(A))
    emis = np.empty(A, np.float32)
    for s, ch in enumerate(order):
        emis[s] = abs(w2[ch]) if s < 8 else w2[ch]
    return order, emis, tuple(groups)


def _host_prep(positions, W1, b1, W2, b2):
    pos = np.asarray(positions, dtype=np.float32)
    W1 = np.asarray(W1, dtype=np.float32)
    b1 = np.asarray(b1, dtype=np.float32)
    W2 = np.asarray(W2, dtype=np.float32)
    b2 = np.asarray(b2, dtype=np.float32)

    W1p, W1d = W1[:C], W1[C:]
    w2 = W2[:, 0]
    order, emis, groups = _plan_channels(w2)

    Wu = (W1p - W1d)[:, order] * emis
    Wd = W1d[:, order] * emis
    b1v = b1[order] * emis

    u = (pos @ Wu + b1v).astype(f16)     # [B,T,N,A]
    v = (pos @ Wd).astype(f16)

    # vT rows: 0 = ones (u row), 1:17 = v channels, 17:27 = dist lhsT.
    # Rows 27:128 are zeroed on device (strip broadcast), so only 27 rows
    # ship over DMA.  The dist matmul is folded into the H matmul as
    # columns 2048:2176 (z[j,i] per t).
    vT = np.zeros((B, 27, T * N), dtype=f16)
    vT[:, 0] = np.asarray(1.0, dtype=f16)
    vT[:, 1:1 + A] = v.transpose(0, 3, 1, 2).reshape(B, A, T * N)

    # channel-major moving row: col = a*N + i; z cols 2048:2176 are zero
    uflat = np.zeros((B, T, 1, NAZ), dtype=f16)
    uflat[:, :, 0, :NA] = u.transpose(0, 1, 3, 2).reshape(B, T, NA)

    delta = np.zeros((16, NAZ), dtype=f16)
    for a in range(A):
        delta[a, a * N:(a + 1) * N] = np.asarray(1.0, dtype=f16)

    # dist matmul operands: z = KAPPA*(R2-d2), sqrt(KAPPA) per side
    sk = np.sqrt(KAPPA)
    pos64 = pos.astype(np.float64)
    n2 = (pos64 ** 2).sum(-1)
    px = pos64[..., 0].reshape(B, T * N)
    py = pos64[..., 1].reshape(B, T * N)
    n2f = n2.reshape(B, T * N)

    def hilo(x):
        hi = x.astype(f16)
        lo = (x - hi.astype(np.float64)).astype(f16)
        return hi, lo

    pxh, pxl = hilo(sk * px)
    pyh, pyl = hilo(sk * py)
    n2jh, n2jl = hilo(-sk * n2f)
    p2xh, p2xl = hilo(2 * sk * px)
    p2yh, p2yl = hilo(2 * sk * py)
    n2ih, n2il = hilo(sk * (R2 - n2f))
    skones = np.full_like(pxh, sk)
    lhsTd = np.stack([pxh, pxh, pxl, pyh, pyh, pyl, skones, skones,
                      n2jh, n2jl], axis=1).astype(f16)
    rhsd = np.stack([p2xh, p2xl, p2xh, p2yh, p2yl, p2yh, n2ih, n2il,
                     skones, skones], axis=1).astype(f16)
    vT[:, 17:27] = lhsTd
    rhsdS = np.ascontiguousarray(
        rhsd.reshape(B, 10, T, N).transpose(0, 2, 1, 3))  # [B,T,10,N]

    pos16 = pos.astype(f16)
    pos3 = np.empty((B, N, T * 3), f16)
    p3 = pos3.reshape(B, N, T, 3)
    p3[..., 0] = pos16[..., 0].transpose(0, 2, 1)
    p3[..., 1] = pos16[..., 1].transpose(0, 2, 1)
    p3[..., 2] = 1.0

    posI = np.empty((B, N, T * 2), np.float32)
    pI = posI.reshape(B, N, T, 2)
    pI[..., 0] = pos16[..., 0].astype(np.float32).transpose(0, 2, 1)
    pI[..., 1] = pos16[..., 1].astype(np.float32).transpose(0, 2, 1)

    ident = np.eye(N, dtype=f16)

    return dict(vT=vT, uflat=uflat, delta=delta, rhsdS=rhsdS,
                pos3=pos3, posI=posI, ident=ident, groups=groups,
                b2=float(b2[0]))


def _build_program(groups, b2val):
    import concourse.bacc as bacc
    import concourse.mybir as mybir
    import concourse.tile as tile

    f32 = mybir.dt.float32
    fp16 = mybir.dt.float16
    Alu = mybir.AluOpType
    Act = mybir.ActivationFunctionType

    K2 = N

    nc = bacc.Bacc()

    vT_p = nc.declare_dram_parameter("vT", [27, T * N], fp16, isOutput=False)
    uflat_p = nc.declare_dram_parameter("uflat", [T, 1, NAZ], fp16,
                                        isOutput=False)
    delta_p = nc.declare_dram_parameter("delta", [16, NAZ], fp16,
                                        isOutput=False)
    rhsdS_p = nc.declare_dram_parameter("rhsdS", [T, 10, N], fp16,
                                        isOutput=False)
    pos3_p = nc.declare_dram_parameter("pos3", [N, T * 3], fp16,
                                       isOutput=False)
    posI_p = nc.declare_dram_parameter("posI", [N, T * 2], f32,
                                       isOutput=False)
    ident_p = nc.declare_dram_parameter("ident", [N, N], fp16,
                                        isOutput=False)
    out_p = nc.declare_dram_parameter("out", [T, N, C], f32, isOutput=True)

    with tile.TileContext(nc) as tc:
        with (
            tc.tile_pool(name="pers", bufs=1) as pers,
            tc.tile_pool(name="hpsum", bufs=5, space="PSUM") as hpsum,
            tc.tile_pool(name="spsum", bufs=2, space="PSUM") as spsum,
            tc.tile_pool(name="fpsum", bufs=1, space="PSUM") as fpsum,
            tc.tile_pool(name="rwork", bufs=3) as rwork,
            tc.tile_pool(name="qwork", bufs=4) as qwork,
            tc.tile_pool(name="awork", bufs=4) as awork,
            tc.tile_pool(name="twork", bufs=2) as twork,
        ):
            vT_s = pers.tile([K2, T * N], fp16, tag="vT")
            pos3_s = pers.tile([N, T * 3], fp16, tag="pos3")
            posI_s = pers.tile([N, T * 2], f32, tag="posI")
            ident_s = pers.tile([N, N], fp16, tag="ident")
            bias_s = pers.tile([N, 1], f32, tag="bias")
            nc.gpsimd.memset(bias_s[:], b2val)
            rhH = [pers.tile([K2, NAZ], fp16, tag=f"rh{i}", name=f"rh{i}")
                   for i in range(8)]

            # Engine memsets must start at partition 0: zero each buffer
            # fully first, then DMA the data rows over it.  Chunked and
            # alternated DVE/Pool so rhH[0]/vT[...t=0] are ready early.
            nc.gpsimd.dma_start(pos3_s[:], pos3_p[:])
            nc.gpsimd.dma_start(posI_s[:], posI_p[:])
            nc.gpsimd.dma_start(ident_s[:], ident_p[:])
            u32 = mybir.dt.uint32
            for i in range(8):
                eng = nc.vector if i % 2 == 0 else nc.gpsimd
                eng.memset(rhH[i][:].bitcast(u32), 0)
                nc.gpsimd.dma_start(rhH[i][1:17, :], delta_p[:])
                eng2 = nc.gpsimd if i % 2 == 0 else nc.vector
                eng2.memset(vT_s[:, 1024 * i:1024 * (i + 1)].bitcast(u32), 0)
                nc.gpsimd.dma_start(vT_s[0:27, 1024 * i:1024 * (i + 1)],
                                    vT_p[:, 1024 * i:1024 * (i + 1)])

            for pi in range(4):
                nc.sync.dma_start(rhH[pi][0:1, :], uflat_p[pi])
                nc.sync.dma_start(rhH[pi][17:27, NA:NAZ], rhsdS_p[pi])

            pd = None
            pairs = {}
            fp_tiles = {}
            qtiles = {}
            rtiles = {}

            def emit_accum_half(p, g):
                """Sum Q(2p+g) slots into S_pe[p][:, g*N:(g+1)*N] on PE."""
                st = pairs[p]
                q = qtiles.pop(2 * p + g)
                for k in range(8):
                    nc.tensor.matmul(st["sp"][:, (2 + g) * N:(3 + g) * N],
                                     ident_s[:],
                                     q[:, k * N:(k + 1) * N],
                                     start=(k == 0), stop=(k == 7))

            def emit_attsig(p):
                st = pairs[p]
                att2 = awork.tile([N, 2 * N], fp16, tag="att2")
                nc.scalar.activation(att2[:], st["sp"][:, 2 * N:4 * N],
                                     Act.Sigmoid, bias=bias_s[:], scale=1.0)
                st["att2"] = att2

            def emit_wmul(p):
                st = pairs[p]
                w2t = awork.tile([N, 2 * N], fp16, tag="w2t")
                nc.gpsimd.tensor_mul(w2t[:], st["att2"][:], st["m2"][:])
                st["w2"] = w2t

            def emit_finals(p):
                st = pairs[p]
                if (p % 4) == 0:
                    fp_tiles[p // 4] = fpsum.tile([N, 4 * TG], f32, tag="F",
                                                  name="F")
                pf = fp_tiles[p // 4]
                st["pf"] = pf
                for gg in range(2):
                    tt = 2 * p + gg
                    g8 = tt % TG
                    s = gg * N
                    nc.tensor.matmul(pf[:, 4 * g8:4 * g8 + 3],
                                     st["w2"][:, s:s + N],
                                     pos3_s[:, 3 * tt:3 * tt + 3],
                                     start=True, stop=True)
                    nc.tensor.matmul(pf[:, 4 * g8 + 3:4 * g8 + 4],
                                     st["m2"][:, s:s + N],
                                     pos3_s[:, 3 * tt + 2:3 * tt + 3],
                                     start=True, stop=True)

            def emit_tail(g):
                pf = fp_tiles[g]
                t0 = g * TG
                pf3 = pf[:].rearrange("p (g c) -> p g c", c=4)
                pI3 = posI_s[:, 2 * t0:2 * (t0 + TG)].rearrange(
                    "p (g c) -> p g c", c=2)
                cnt8 = twork.tile([N, 8], f32, tag="cnt8")
                rcp8 = twork.tile([N, 8], f32, tag="rcp8")
                sw8 = twork.tile([N, 16], f32, tag="sw8")
                outst = twork.tile([N, 16], f32, tag="outst")
                nc.vector.tensor_scalar(cnt8[:], pf3[:, :, 3], -1.0, 1e-6,
                                        op0=Alu.add, op1=Alu.max)
                nc.vector.reciprocal(rcp8[:], cnt8[:])
                s3 = sw8[:].rearrange("p (g c) -> p g c", c=2)
                o3 = outst[:].rearrange("p (g c) -> p g c", c=2)
                for c in range(2):
                    nc.vector.tensor_mul(s3[:, :, c], pf3[:, :, 2],
                                         pI3[:, :, c])
                    nc.vector.tensor_sub(o3[:, :, c], pf3[:, :, c],
                                         s3[:, :, c])
                    nc.vector.tensor_mul(o3[:, :, c], o3[:, :, c], rcp8[:])
                nc.gpsimd.dma_start(
                    out_p[t0:t0 + TG].rearrange("t n c -> n t c"), outst[:])

            alu = {"max": Alu.max, "min": Alu.min,
                   "add": Alu.add, "subtract": Alu.subtract}

            for t in range(T):
                g2 = t % 2
                g8 = t % TG
                p_cur = t // 2
                rh = rhH[t % 8]
                if t + 4 < T:
                    nc.sync.dma_start(rhH[(t + 4) % 8][0:1, :],
                                      uflat_p[t + 4])
                    nc.sync.dma_start(rhH[(t + 4) % 8][17:27, NA:NAZ],
                                      rhsdS_p[t + 4])

                # H matmuls: hp1/hp2 = ACT slots 0:8, hp3/hp4 = DVE 8:16
                hp = [hpsum.tile([N, 512], f32, tag="H", name=f"hp{c}")
                      for c in range(4)]
                lhs = vT_s[:, t * N:(t + 1) * N]
                for c in range(4):
                    nc.tensor.matmul(hp[c][:], lhs,
                                     rh[:, 512 * c:512 * (c + 1)],
                                     start=True, stop=True)

                # z matmul (dist folded into the H family)
                if g2 == 0:
                    sp = spsum.tile([N, 4 * N], f32, tag="S", name="sp")
                    pd = sp
                    pairs[p_cur] = dict(sp=sp)
                nc.tensor.matmul(pd[:, g2 * N:(g2 + 1) * N],
                                 lhs, rh[:, NA:NAZ],
                                 start=True, stop=True)

                if g2 == 0 and p_cur >= 2:
                    emit_finals(p_cur - 2)
                    pairs.pop(p_cur - 2)

                # PE: accum of previous pair's Q last (freshest DVE dep)
                if p_cur >= 1:
                    emit_accum_half(p_cur - 1, g2)

                # ACT: one relu evac of slots 0:8
                ract = rwork.tile([N, HALF], fp16, tag="ract")
                nc.scalar.activation(ract[:, 0:512], hp[0][:], Act.Relu)
                nc.scalar.activation(ract[:, 512:1024], hp[1][:], Act.Relu)
                rtiles[t] = ract

                if g2 == 1:
                    st = pairs[p_cur]
                    m2t = awork.tile([N, 2 * N], fp16, tag="m2t")
                    nc.scalar.activation(m2t[:], pd[:, 0:2 * N],
                                         Act.Sigmoid, scale=16.0)
                    st["m2"] = m2t
                    if p_cur >= 1:
                        emit_attsig(p_cur - 1)

                # DVE: fused STT evacs of slots 8:16 into Q
                q = qwork.tile([N, HALF], fp16, tag="Q")
                qtiles[t] = q
                for (g0, g1, op0, op1) in groups:
                    for (s0, s1) in ((g0, min(g1, 4)), (max(g0, 4), g1)):
                        if s0 >= s1:
                            continue
                        hpb = hp[2] if s1 <= 4 else hp[3]
                        b0 = s0 - (0 if s1 <= 4 else 4)
                        nc.vector.scalar_tensor_tensor(
                            q[:, s0 * N:s1 * N],
                            hpb[:, b0 * N:(b0 + s1 - s0) * N], 0.0,
                            ract[:, s0 * N:s1 * N],
                            op0=alu[op0], op1=alu[op1])
                rtiles.pop(t)

                if g2 == 1 and p_cur >= 1:
                    emit_wmul(p_cur - 1)

                if g2 == 1 and g8 == 3 and t >= 11:
                    emit_tail((t - 11) // 8)

            # epilogue
            P = T // 2
            emit_accum_half(P - 1, 0)
            emit_accum_half(P - 1, 1)
            emit_attsig(P - 1)
            emit_wmul(P - 1)
            emit_finals(P - 2)
            emit_finals(P - 1)
            emit_tail(NG - 1)

    nc.compile()
    return nc


def kernel(positions, W1, b1, W2, b2, _trace=False, _trace_kwargs=None):
    from concourse.bass_utils import run_bass_kernel_spmd

    prep = _host_prep(positions, W1, b1, W2, b2)
    key = (prep["groups"], prep["b2"])
    if key not in _CACHE:
        _CACHE[key] = _build_program(prep["groups"], prep["b2"])
    nc = _CACHE[key]

    in_maps = []
    for b in range(B):
        in_maps.append({
            "vT": np.ascontiguousarray(prep["vT"][b]),
            "uflat": np.ascontiguousarray(prep["uflat"][b]),
            "delta": prep["delta"],
            "rhsdS": np.ascontiguousarray(prep["rhsdS"][b]),
            "pos3": np.ascontiguousarray(prep["pos3"][b]),
            "posI": np.ascontiguousarray(prep["posI"][b]),
            "ident": prep["ident"],
        })

    kw = {}
    if _trace:
        kw["trace"] = True
        if _trace_kwargs:
            kw.update(_trace_kwargs)
    res = run_bass_kernel_spmd(nc, in_maps, list(range(B)), **kw)
    out = np.stack([r["out"] for r in res.results], axis=0).astype(np.float32)
    if _trace:
        return out, res
    return out
